# revision 1
# baseline (speedup 1.0000x reference)
"""Multi-head attention (dense_transformer) Trainium2 Bass kernel.

Problem: x[8, 512, 32, 32]; per-batch 1x1-conv QKV projections, 8-head
attention over N=H*W=1024 positions (head_dim 64), output projection,
residual. Sharding: data-parallel over batch B=8 across the 8 cores —
one batch element per core, no collectives.

Per-core dataflow (all matmul inputs bf16, accumulation fp32):
  - Host pre-transposes weights to [c, o] layout and pre-casts to bf16.
  - Q, K in [c, n] layout: Q[ot] = WqT[ct].T @ x16[ct] (+bq).
  - V kept transposed: VT[jt][n, o] = x16[:, jt].T @ WvT (+bv), stored
    per-head with a ones column appended: [128, 8 heads, 65].
  - S^T[j, i] = K_h.T Q_h per head: j on partitions -> AV matmul needs
    no transposes anywhere. exp via ScalarE with the 1/sqrt(64) scale
    folded in; softmax denominator comes from the VT ones column during
    the AV matmul (PSUM row 64); normalization = reciprocal + DRAM-
    bounce partition broadcast + VectorE multiply.
  - out = WoT.T @ O + (x32 + bo prefolded), DMA'd out in fp32.

PSUM (8 banks) is phase-scoped: projections use a 4-buf half-bank pool
that closes before the AV-accumulator pool opens in the same banks.
"""

import sys

if "/opt/trn_rl_repo" not in sys.path:
    sys.path.insert(0, "/opt/trn_rl_repo")

import numpy as np
import ml_dtypes

import concourse.bass as bass
import concourse.mybir as mybir
from concourse.tile import TileContext

DIM = 512
NH = 8
HD = 64
N = 1024
P = 128
CT = DIM // P  # 4 c-tiles of 128 channels
JT = N // P    # 8 j-tiles of 128 positions
F32 = mybir.dt.float32
BF16 = mybir.dt.bfloat16
AOP = mybir.AluOpType
EXP = mybir.ActivationFunctionType.Exp


class FixedTileContext(TileContext):
    """Works around a walrus/bass snapshot mismatch: this walrus build
    accepts only one sync-wait command per instruction, but Tile's wait
    assigner happily attaches several. After scheduling, excess waits on
    any instruction are peeled off onto same-engine NOPs inserted right
    before it (same blocking semantics: the engine executes in order)."""

    MAX_WAITS = 1
    MAX_WAITS_DATA = 1
    _wsplit_ctr = 0

    def _split_sync_waits(self):
        seq_only = mybir.SEQUENCER_ONLY_OPCODES
        for fn in self.nc.m.functions:
            for blk in fn.blocks:
                insts = list(blk.instructions)
                out = []
                for inst in insts:
                    si = inst.sync_info
                    limit = (
                        self.MAX_WAITS
                        if inst.opcode in seq_only
                        else self.MAX_WAITS_DATA
                    )
                    if si is not None and len(si.on_wait) > limit:
                        waits = list(si.on_wait)
                        movers = waits[:-limit]
                        keep = waits[-limit:]
                        del si.on_wait[:]
                        for w in keep:
                            si.on_wait.append(w)
                        for w in movers:
                            FixedTileContext._wsplit_ctr += 1
                            nop = mybir.InstNoOp(
                                name=f"wsplit-{FixedTileContext._wsplit_ctr}",
                                ins=[],
                                outs=[],
                            )
                            nop.engine = inst.engine
                            nop.sync_info = mybir.SyncInfo(on_wait=[w], on_update=[])
                            out.append(nop)
                    out.append(inst)
                if len(out) != len(insts):
                    del blk.instructions[:]
                    for i in out:
                        blk.add_instruction(i)

    split_on_exit = True

    def __exit__(self, *exc):
        ret = super().__exit__(*exc)
        if exc[0] is None and self.split_on_exit:
            self._split_sync_waits()
        return ret


def build_nc(split_waits=True):
    nc = bass.Bass()

    x32d = nc.dram_tensor("x32", [DIM, N], F32, kind="ExternalInput")
    x16d = nc.dram_tensor("x16", [DIM, N], BF16, kind="ExternalInput")
    wqd = nc.dram_tensor("wqt", [DIM, DIM], BF16, kind="ExternalInput")
    wkd = nc.dram_tensor("wkt", [DIM, DIM], BF16, kind="ExternalInput")
    wvd = nc.dram_tensor("wvt", [DIM, DIM], BF16, kind="ExternalInput")
    wod = nc.dram_tensor("wot", [DIM, DIM], BF16, kind="ExternalInput")
    bqd = nc.dram_tensor("bq", [DIM], F32, kind="ExternalInput")
    bkd = nc.dram_tensor("bk", [DIM], F32, kind="ExternalInput")
    bvd = nc.dram_tensor("bv", [DIM], F32, kind="ExternalInput")
    bod = nc.dram_tensor("bo", [DIM], F32, kind="ExternalInput")
    outd = nc.dram_tensor("out", [DIM, N], F32, kind="ExternalOutput")

    FixedTileContext.split_on_exit = split_waits
    with FixedTileContext(nc) as tc:
        with (
            tc.tile_pool(name="persist", bufs=1) as persist,
            tc.tile_pool(name="ppool", bufs=32) as ppool,
            tc.tile_pool(name="small", bufs=3) as small,
            tc.tile_pool(name="otile", bufs=8) as otile,
            tc.tile_pool(name="dram", bufs=1, space="DRAM") as dram,
            tc.tile_pool(name="psS", bufs=2, space="PSUM") as psS_pool,
        ):
            # weights/biases ride ScalarE's DMA queues (ScalarE is idle
            # until the first exp) so they don't serialize behind the x
            # loads on SP's queues
            def load_w(wd, name):
                wr = wd.rearrange("(t p) o -> t p o", p=P)
                ws = []
                for t in range(CT):
                    wt = persist.tile(
                        [P, DIM], BF16, tag=f"{name}_{t}", name=f"{name}_{t}"
                    )
                    nc.scalar.dma_start(out=wt, in_=wr[t])
                    ws.append(wt)
                return ws

            def load_b(bd, name):
                bt = persist.tile([P, CT], F32, tag=name, name=name)
                nc.scalar.dma_start(out=bt, in_=bd.rearrange("(t p) -> p t", p=P))
                return bt

            # S^T + exp for one head pair. Emission alternates PE row
            # groups 0-63 / 64-127 between consecutive matmuls so the
            # hardware overlaps them (per-subarray concurrency) even
            # though K=64 only half-fills the array.
            def s_phase(pair):
                P16 = {}
                for jt in range(JT):
                    tiles = {}

                    def smm(h2, ih):
                        base = 64 * h2
                        nc.tensor.matmul(
                            tiles[h2][:, ih * 512 : (ih + 1) * 512],
                            lhsT=K[pair][base : base + 64, jt * P : (jt + 1) * P],
                            rhs=Q[pair][base : base + 64, ih * 512 : (ih + 1) * 512],
                            start=True,
                            stop=True,
                        )

                    tiles[0] = psS_pool.tile([P, N], F32, tag="psS", name="psS")
                    smm(0, 0)
                    tiles[1] = psS_pool.tile([P, N], F32, tag="psS", name="psS")
                    smm(1, 0)
                    smm(0, 1)
                    smm(1, 1)
                    for h2 in range(2):
                        pt = ppool.tile([P, N], BF16, tag="p16", name="p16")
                        nc.scalar.activation(pt, tiles[h2], EXP, scale=0.125)
                        P16[(jt, h2)] = pt
                return P16

            def p16_slice(P16, jt, h2, ih):
                return P16[(jt, h2)][:, ih * 512 : (ih + 1) * 512]

            # AV matmul + softmax normalization for one head pair. The raw
            # head output is copied out of PSUM right away (frees the psO
            # slot for the next head's AV); the DRAM-bounce broadcast and
            # the normalize multiply then run off the critical PSUM path.
            def av_phase(pair, P16, psO_pool, O16, rdram):
                last_pair = pair == NH // 2 - 1
                h2_order = (1, 0) if last_pair else (0, 1)
                for h2 in h2_order:
                    h = 2 * pair + h2
                    rec = small.tile([HD + 1, N], F32, tag="rec", name="rec")
                    oraw = small.tile([HD, N], F32, tag="oraw", name="oraw")
                    rb = small.tile([HD, N], F32, tag="rb", name="rb")
                    for ih in range(2):
                        sl = slice(ih * 512, (ih + 1) * 512)
                        po = psO_pool.tile([HD + 1, 512], F32, tag="psO", name="po")
                        for jt in range(JT):
                            nc.tensor.matmul(
                                po,
                                lhsT=VT[jt][:, h, :],
                                rhs=p16_slice(P16, jt, h2, ih),
                                start=(jt == 0),
                                stop=(jt == JT - 1),
                            )
                        # softmax denominator sits in row HD of po
                        nc.vector.reciprocal(rec[HD : HD + 1, sl], po[HD : HD + 1, :])
                        # copy the raw head output out of PSUM immediately
                        # (frees the psO slot); on the last pair ScalarE is
                        # done with exps, so use it and keep DVE off the
                        # critical chain
                        if last_pair:
                            nc.scalar.copy(oraw[:, sl], po[0:HD, :])
                        else:
                            nc.vector.tensor_copy(oraw[:, sl], po[0:HD, :])
                        # per-half DRAM bounce broadcasts 1/colsum across
                        # partitions (SBUF APs reject 0 partition stride)
                        dmae = nc.scalar if last_pair else nc.sync
                        dmae.dma_start(
                            out=rdram[h : h + 1, sl], in_=rec[HD : HD + 1, sl]
                        )
                        rsrc = rdram[h : h + 1, sl]
                        nc.sync.dma_start(
                            out=rb[:, sl],
                            in_=bass.AP(
                                tensor=rsrc.tensor,
                                offset=rsrc.offset,
                                ap=[[0, HD]] + list(rsrc.ap[1:]),
                            ),
                        )
                    osc = None
                    if h2 != 0:
                        osc = small.tile([HD, N], BF16, tag="osc", name="osc")
                    for ih in range(2):
                        sl = slice(ih * 512, (ih + 1) * 512)
                        if h2 == 0:
                            nc.vector.tensor_tensor(
                                O16[pair][0:HD, sl], oraw[:, sl], rb[:, sl], AOP.mult
                            )
                        else:
                            nc.vector.tensor_tensor(
                                osc[:, sl], oraw[:, sl], rb[:, sl], AOP.mult
                            )
                            (nc.scalar if last_pair else nc.sync).dma_start(
                                out=O16[pair][HD:P, sl], in_=osc[:, sl]
                            )

            with tc.tile_pool(name="pp", bufs=4, space="PSUM") as pp:
                # ---------- input loads ----------
                x16r = x16d.rearrange("(t p) n -> t p n", p=P)
                xs16 = []
                for t in range(CT):
                    xt = persist.tile([P, N], BF16, tag=f"x16_{t}", name=f"x16_{t}")
                    nc.sync.dma_start(out=xt, in_=x16r[t])
                    xs16.append(xt)

                # interleave wq/wk tiles so K0's accumulation matmuls can
                # trickle-start alongside Q0's instead of waiting for the
                # whole of wq to finish on the same queue
                wqr = wqd.rearrange("(t p) o -> t p o", p=P)
                wkr = wkd.rearrange("(t p) o -> t p o", p=P)
                wqs, wks = [], []
                for t in range(CT):
                    wqt_ = persist.tile([P, DIM], BF16, tag=f"wq_{t}", name=f"wq_{t}")
                    nc.scalar.dma_start(out=wqt_, in_=wqr[t])
                    wqs.append(wqt_)
                    wkt_ = persist.tile([P, DIM], BF16, tag=f"wk_{t}", name=f"wk_{t}")
                    nc.scalar.dma_start(out=wkt_, in_=wkr[t])
                    wks.append(wkt_)
                bq_sb = load_b(bqd, "bq")
                bk_sb = load_b(bkd, "bk")

                # trigger the ~2.7us exp table load on ScalarE right after
                # its weight-DMA issues, so the first real exp doesn't pay it
                warm = small.tile([1, 8], F32, tag="warm", name="warm")
                nc.vector.memset(warm, 0.0)
                nc.scalar.activation(warm, warm, EXP)

                # ------ Q, K projections: [CT][128, N] bf16, [c, n] layout
                def project_one(ws, b_sb, name, ot):
                    qt = persist.tile(
                        [P, N], BF16, tag=f"{name}_{ot}", name=f"{name}_{ot}"
                    )
                    for nh in range(2):
                        ps = pp.tile(
                            [P, 512], F32, tag="pp", name=f"pp_{name}{ot}{nh}"
                        )
                        for ct in range(CT):
                            nc.tensor.matmul(
                                ps,
                                lhsT=ws[ct][:, ot * P : (ot + 1) * P],
                                rhs=xs16[ct][:, nh * 512 : (nh + 1) * 512],
                                start=(ct == 0),
                                stop=(ct == CT - 1),
                            )
                        nc.vector.tensor_scalar_add(
                            qt[:, nh * 512 : (nh + 1) * 512],
                            ps,
                            b_sb[:, ot : ot + 1],
                        )
                    return qt

                Q, K = [], []
                Q.append(project_one(wqs, bq_sb, "q", 0))
                K.append(project_one(wks, bk_sb, "k", 0))

                # pair 0's S^T + exp right away: gets ScalarE going while
                # the remaining projections stream on the PE
                P16_0 = s_phase(0)
                Q.append(project_one(wqs, bq_sb, "q", 1))
                K.append(project_one(wks, bk_sb, "k", 1))
                P16_1 = s_phase(1)

                # ------ V^T projection: VT[jt] = [128, NH, HD+1] bf16
                wvs = load_w(wvd, "wv")
                bvB = persist.tile([P, DIM], F32, tag="bvB", name="bvB")
                nc.gpsimd.dma_start(
                    out=bvB,
                    in_=bass.AP(
                        tensor=bvd[:].tensor, offset=0, ap=[[0, P], [1, DIM]]
                    ),
                )
                VT = []
                for jt in range(JT):
                    vt = persist.tile(
                        [P, NH, HD + 1], BF16, tag=f"vt_{jt}", name=f"vt_{jt}"
                    )
                    ps = pp.tile([P, 512], F32, tag="pp", name=f"pp_v{jt}")
                    for ct in range(CT):
                        nc.tensor.matmul(
                            ps,
                            lhsT=xs16[ct][:, jt * P : (jt + 1) * P],
                            rhs=wvs[ct],
                            start=(ct == 0),
                            stop=(ct == CT - 1),
                        )
                    nc.vector.tensor_tensor(
                        vt[:, :, 0:HD],
                        ps.rearrange("p (h d) -> p h d", h=NH),
                        bvB.rearrange("p (h d) -> p h d", h=NH),
                        AOP.add,
                    )
                    nc.vector.memset(vt[:, :, HD : HD + 1], 1.0)
                    VT.append(vt)

                for ot in range(2, CT):
                    Q.append(project_one(wqs, bq_sb, "q", ot))
                    K.append(project_one(wks, bk_sb, "k", ot))

            # ---------- attention (heads 2p / 2p+1 live on partitions
            # 0-63 / 64-127 of Q/K c-tile p); the AV-accumulator pool
            # reuses banks the projection pool just released
            O16 = [
                persist.tile([P, N], BF16, tag=f"o16_{t}", name=f"o16_{t}")
                for t in range(CT)
            ]
            rdram = dram.tile([NH, N], F32, tag="rdram", name="rdram")
            with tc.tile_pool(name="psO", bufs=4, space="PSUM") as psO_pool:
                av_phase(0, P16_0, psO_pool, O16, rdram)
                P16_2 = s_phase(2)
                av_phase(1, P16_1, psO_pool, O16, rdram)
                P16_3 = s_phase(3)
                av_phase(2, P16_2, psO_pool, O16, rdram)
                av_phase(3, P16_3, psO_pool, O16, rdram)

                # loads for the output projection (low priority; the DMA
                # queues have slack mid-kernel)
                wos = load_w(wod, "wo")
                bo_sb = load_b(bod, "bo")
                x32r = x32d.rearrange("(t p) n -> t p n", p=P)
                xs32 = []
                for t in range(CT):
                    xt32 = persist.tile(
                        [P, N], F32, tag=f"x32_{t}", name=f"x32_{t}"
                    )
                    nc.sync.dma_start(out=xt32, in_=x32r[t])
                    nc.vector.tensor_scalar_add(xt32, xt32, bo_sb[:, t : t + 1])
                    xs32.append(xt32)

            # ---------- output projection + residual. ot0/ot1 psum tiles
            # come from the psS pool (slots drained by pair-3 exps);
            # ot2/ot3 from a pool reusing the psO banks (drained by the
            # early PSUM copies) — all 24 ct0-2 matmuls can therefore run
            # while the last head's epilogue is still in flight.
            with tc.tile_pool(name="po3", bufs=2, space="PSUM") as po3:
                outr = outd.rearrange("(t p) n -> t p n", p=P)

                def op_pre(ot, pool=None):
                    # ct 0..2 accumulation: issuable while the last head
                    # pair (feeding O16[3]) is still in its epilogue
                    if pool is None:
                        ps = psS_pool.tile([P, N], F32, tag="psS", name=f"ps_o{ot}")
                    else:
                        ps = pool.tile([P, N], F32, tag="op34", name=f"ps_o{ot}")
                    for nh in range(2):
                        for ct in range(CT - 1):
                            nc.tensor.matmul(
                                ps[:, nh * 512 : (nh + 1) * 512],
                                lhsT=wos[ct][:, ot * P : (ot + 1) * P],
                                rhs=O16[ct][:, nh * 512 : (nh + 1) * 512],
                                start=(ct == 0),
                                stop=(ct == CT - 2),
                            )
                    return ps

                def op_post(ot, ps):
                    # ct 3 continues the accumulation in a second group,
                    # then bias+residual and writeback
                    for nh in range(2):
                        nc.tensor.matmul(
                            ps[:, nh * 512 : (nh + 1) * 512],
                            lhsT=wos[CT - 1][:, ot * P : (ot + 1) * P],
                            rhs=O16[CT - 1][:, nh * 512 : (nh + 1) * 512],
                            start=False,
                            stop=True,
                            skip_group_check=True,
                        )
                    for nh in range(2):
                        ob = otile.tile([P, 512], F32, tag="ob", name="ob")
                        nc.vector.tensor_tensor(
                            ob,
                            ps[:, nh * 512 : (nh + 1) * 512],
                            xs32[ot][:, nh * 512 : (nh + 1) * 512],
                            AOP.add,
                        )
                        nc.sync.dma_start(
                            out=outr[ot][:, nh * 512 : (nh + 1) * 512], in_=ob
                        )

                ps0 = op_pre(0)
                ps1 = op_pre(1)
                ps2 = op_pre(2, po3)
                ps3 = op_pre(3, po3)
                op_post(0, ps0)
                op_post(1, ps1)
                op_post(2, ps2)
                op_post(3, ps3)
    return nc


_BF = ml_dtypes.bfloat16


def _prep_maps(x, Wq, bq, Wk, bk, Wv, bv, Wo, bo):
    # plain numpy up front: inputs may arrive as jax device arrays and
    # transforming those would trigger on-device jax execution
    x, Wq, bq, Wk, bk, Wv, bv, Wo, bo = (
        np.asarray(a) for a in (x, Wq, bq, Wk, bk, Wv, bv, Wo, bo)
    )
    B, C, H, W = x.shape
    xf = np.ascontiguousarray(x.reshape(B, C, H * W)).astype(np.float32)
    shared = {
        "wqt": np.ascontiguousarray(Wq.T).astype(_BF),
        "wkt": np.ascontiguousarray(Wk.T).astype(_BF),
        "wvt": np.ascontiguousarray(Wv.T).astype(_BF),
        "wot": np.ascontiguousarray(Wo.T).astype(_BF),
        "bq": np.asarray(bq, np.float32),
        "bk": np.asarray(bk, np.float32),
        "bv": np.asarray(bv, np.float32),
        "bo": np.asarray(bo, np.float32),
    }
    in_maps = []
    for b in range(B):
        m = dict(shared)
        m["x32"] = xf[b]
        m["x16"] = xf[b].astype(_BF)
        in_maps.append(m)
    return in_maps


def kernel(x, Wq, bq, Wk, bk, Wv, bv, Wo, bo, _trace=False):
    from concourse.bass_utils import run_bass_kernel_spmd

    x = np.asarray(x)
    B, C, H, W = x.shape
    in_maps = _prep_maps(x, Wq, bq, Wk, bk, Wv, bv, Wo, bo)
    nc = build_nc()
    res = run_bass_kernel_spmd(nc, in_maps, core_ids=list(range(B)), trace=_trace)
    out = np.stack([res.results[b]["out"] for b in range(B)])
    out = out.reshape(B, C, H, W).astype(np.float32)
    if _trace:
        kernel.last_results = res
    return out



# revision 6
# speedup vs baseline: 1.0105x; 1.0105x over previous
"""Multi-head attention (dense_transformer) Trainium2 Bass kernel.

Problem: x[8, 512, 32, 32]; per-batch 1x1-conv QKV projections, 8-head
attention over N=H*W=1024 positions (head_dim 64), output projection,
residual. Sharding: data-parallel over batch B=8 across the 8 cores --
one batch element per core, no collectives.

Key design points (v2, fp8-DoubleRow rewrite):
  - All projection / AV / output matmuls run in fp8e4m3 DoubleRow mode:
    one instruction contracts two 128-deep k-tiles at 0.5 cycles/row,
    4x the throughput of the bf16 accumulation chains it replaces.
  - Bias algebra: bk is dropped outright (softmax is shift-invariant
    along j, so the K bias cancels exactly); bv is folded into bo on the
    host (bo' = bo + Wo@bv, exact since attention rows sum to 1); bo'
    rides in the precomputed residual tensor xr = x + bo'; bq is added
    by a tiny [1-partition] DoubleRow ones-row matmul into the Q psum.
    No per-element bias vector ops remain on the device.
  - Q/K stay bf16 for the S = K^T Q matmuls (contraction is only 64
    deep -- fp8 DoubleRow does not apply -- and bf16 runs at the same
    1 cycle/row while keeping S accurate).
  - The softmax exp of the [1024, 1024] S matrix per head (8.4M
    elements -- the single largest engine workload) is split between
    the Activation engine (exact exp -> fp8 out) and the Vector engine
    (a one-instruction Schraudolph bit-trick: int8(S*A + B) aliased as
    fp8e4m3 approximates exp(S/8) to ~3% RMS, well inside the 2e-2
    tolerance).
  - Softmax denominators come free as a 65th ones-column of V^T in the
    AV matmul; gpsimd broadcasts the psum denominator row across 64
    partitions and the normalization is a single tensor_tensor divide
    (no reciprocal ops, no DRAM bounce).
  - GPSIMD (Pool) engine carries V-copies, denominator broadcasts and
    part of the normalize/residual work under the 'proxy' library.
"""

import sys

if "/opt/trn_rl_repo" not in sys.path:
    sys.path.insert(0, "/opt/trn_rl_repo")

import numpy as np
import ml_dtypes

import concourse.bass as bass
import concourse.mybir as mybir
from concourse.tile import TileContext
from concourse import library_config

DIM = 512
NH = 8
HD = 64
N = 1024
P = 128
CT = DIM // P   # 4 c-tiles of 128 channels
JT = N // P     # 8 j-tiles of 128 positions
F32 = mybir.dt.float32
BF16 = mybir.dt.bfloat16
F8 = mybir.dt.float8e4
I8 = mybir.dt.int8
AOP = mybir.AluOpType
EXP = mybir.ActivationFunctionType.Exp
DRM = mybir.MatmulPerfMode.DoubleRow

# Schraudolph exp: fp8e4m3 byte ~= int8(S * SCH_A + SCH_B)  approximates
# exp(S * 0.125). B tuned numerically for truncating float->int casts.
SCH_A = 8.0 / np.log(2.0) * 0.125
SCH_B = 56.08

# (pair, h2, jt) combos whose exp runs on DVE (Schraudolph); rest on ACT.
EXP_DVE = {(p, h2, jt) for p in range(4) for h2 in range(2)
           for jt in (1, 3, 5)} | {(p, 1, 6) for p in (0, 1)}

# engine for each Q/K psum->bf16 copy, by (tensor, ot)
QK_COPY_ENG = {("q", 0): "A", ("k", 0): "V", ("q", 1): "A", ("k", 1): "V",
               ("q", 2): "P", ("k", 2): "P", ("q", 3): "A", ("k", 3): "V"}
# engine for the normalize divide, per head (ACT cannot run tensor_tensor)
NORM_ENG = {0: "P", 1: "V", 2: "P", 3: "V", 4: "P", 5: "V", 6: "V", 7: "P"}
# engine for the residual add, per ot
RESID_ENG = {0: "P", 1: "V", 2: "P", 3: "V"}


class FixedTileContext(TileContext):
    """Works around a walrus/bass snapshot mismatch: this walrus build
    accepts only one sync-wait command per instruction, but Tile's wait
    assigner happily attaches several. After scheduling, excess waits on
    any instruction are peeled off onto same-engine NOPs inserted right
    before it (same blocking semantics: the engine executes in order)."""

    MAX_WAITS = 1
    MAX_WAITS_DATA = 1
    _wsplit_ctr = 0

    def _split_sync_waits(self):
        seq_only = mybir.SEQUENCER_ONLY_OPCODES
        for fn in self.nc.m.functions:
            for blk in fn.blocks:
                insts = list(blk.instructions)
                out = []
                for inst in insts:
                    si = inst.sync_info
                    limit = (
                        self.MAX_WAITS
                        if inst.opcode in seq_only
                        else self.MAX_WAITS_DATA
                    )
                    if si is not None and len(si.on_wait) > limit:
                        waits = list(si.on_wait)
                        movers = waits[:-limit]
                        keep = waits[-limit:]
                        del si.on_wait[:]
                        for w in keep:
                            si.on_wait.append(w)
                        for w in movers:
                            FixedTileContext._wsplit_ctr += 1
                            nop = mybir.InstNoOp(
                                name=f"wsplit-{FixedTileContext._wsplit_ctr}",
                                ins=[],
                                outs=[],
                            )
                            nop.engine = inst.engine
                            nop.sync_info = mybir.SyncInfo(on_wait=[w], on_update=[])
                            out.append(nop)
                    out.append(inst)
                if len(out) != len(insts):
                    del blk.instructions[:]
                    for i in out:
                        blk.add_instruction(i)

    split_on_exit = True

    def __exit__(self, *exc):
        ret = super().__exit__(*exc)
        if exc[0] is None and self.split_on_exit:
            self._split_sync_waits()
        return ret


def build_nc(split_waits=True):
    nc = bass.Bass()

    x8d = nc.dram_tensor("x8", [P, CT, N], F8, kind="ExternalInput")
    xrd = nc.dram_tensor("xr32", [P, CT, N], F32, kind="ExternalInput")
    wq8d = nc.dram_tensor("wq8", [P, CT, DIM], F8, kind="ExternalInput")
    wk8d = nc.dram_tensor("wk8", [P, CT, DIM], F8, kind="ExternalInput")
    wv8d = nc.dram_tensor("wv8", [P, CT, DIM], F8, kind="ExternalInput")
    wo8d = nc.dram_tensor("wo8", [P, CT, DIM], F8, kind="ExternalInput")
    bq8d = nc.dram_tensor("bq8", [1, 2, DIM], F8, kind="ExternalInput")
    on8d = nc.dram_tensor("on8", [1, 2, N], F8, kind="ExternalInput")
    outd = nc.dram_tensor("out", [DIM, N], F32, kind="ExternalOutput")
    outr = outd.rearrange("(t p) n -> t p n", p=P)

    FixedTileContext.split_on_exit = split_waits
    with FixedTileContext(nc) as tc:
        with (
            tc.tile_pool(name="persist", bufs=1) as persist,
            tc.tile_pool(name="small", bufs=2) as small,
            tc.tile_pool(name="obf", bufs=2) as obf,
            tc.tile_pool(name="psS", bufs=2, space="PSUM") as psS,
        ):
            nc.gpsimd.load_library(library_config.proxy)

            # persistent SBUF tensors
            x8 = persist.tile([P, CT, N], F8, tag="x8", name="x8")
            xr32 = persist.tile([P, CT, N], F32, tag="xr32", name="xr32")
            wq8 = persist.tile([P, CT, DIM], F8, tag="wq8", name="wq8")
            wk8 = persist.tile([P, CT, DIM], F8, tag="wk8", name="wk8")
            wv8 = persist.tile([P, CT, DIM], F8, tag="wv8", name="wv8")
            wo8 = persist.tile([P, CT, DIM], F8, tag="wo8", name="wo8")
            bq8 = persist.tile([1, 2, DIM], F8, tag="bq8", name="bq8")
            on8 = persist.tile([1, 2, N], F8, tag="on8", name="on8")
            qt = [persist.tile([P, N], BF16, tag=f"q_{t}", name=f"q_{t}")
                  for t in range(CT)]
            kt = [persist.tile([P, N], BF16, tag=f"k_{t}", name=f"k_{t}")
                  for t in range(CT)]
            vt = [persist.tile([P, 2, NH, HD + 1], F8, tag=f"v_{a}",
                               name=f"v_{a}") for a in range(CT)]
            p8 = {}
            for a in range(CT):
                for h in range(NH):
                    p8[(a, h)] = persist.tile(
                        [P, 2, N], F8, tag=f"p8_{a}_{h}", name=f"p8_{a}_{h}")
            o8 = [persist.tile([P, 2, N], F8, tag=f"o8_{g}", name=f"o8_{g}")
                  for g in range(2)]

            def copy_psum(eng, out_ap, in_ap):
                if eng == "A":
                    nc.scalar.copy(out_ap, in_ap)
                elif eng == "V":
                    nc.vector.tensor_copy(out_ap, in_ap)
                else:
                    nc.gpsimd.tensor_tensor(out_ap, in_ap, in_ap, AOP.max)

            def exp_op(key, dst_ap, src_ap):
                if key in EXP_DVE:
                    nc.vector.tensor_scalar(dst_ap.bitcast(I8), src_ap,
                                            SCH_A, SCH_B, AOP.mult, AOP.add)
                else:
                    nc.scalar.activation(dst_ap, src_ap, EXP, scale=0.125)

            def s_tile(p, h2, jt):
                h = 2 * p + h2
                t = psS.tile([P, N], F32, tag="psS", name=f"s_{p}_{h2}_{jt}")
                for ih in range(2):
                    nc.tensor.matmul(
                        t[:, ih * 512:(ih + 1) * 512],
                        lhsT=kt[p][64 * h2:64 * h2 + 64, jt * P:(jt + 1) * P],
                        rhs=qt[p][64 * h2:64 * h2 + 64, ih * 512:(ih + 1) * 512],
                        start=True, stop=True)
                exp_op((p, h2, jt), p8[(jt // 2, h)][:, jt % 2, :], t)

            with tc.tile_pool(name="pp", bufs=2, space="PSUM") as pp:
                # ---- input DMAs ----
                nc.sync.dma_start(out=x8[:, 0:2, :], in_=x8d[:, 0:2, :])
                nc.gpsimd.dma_start(out=x8[:, 2:4, :], in_=x8d[:, 2:4, :])
                nc.scalar.dma_start(out=wq8, in_=wq8d[:, :, :])
                nc.scalar.dma_start(out=wk8, in_=wk8d[:, :, :])
                nc.scalar.dma_start(out=bq8, in_=bq8d[:, :, :])
                nc.scalar.dma_start(out=on8, in_=on8d[:, :, :])
                nc.sync.dma_start(out=wv8, in_=wv8d[:, :, :])

                # warm the exp table on ACT before the first real exp
                warm = small.tile([1, 8], F32, tag="warm", name="warm")
                nc.vector.memset(warm, 0.0)
                nc.scalar.activation(warm, warm, EXP)

                def proj_qk(w8, ot, with_bias, eng, name):
                    ps = pp.tile([P, N], F32, tag="pp", name=f"pp_{name}{ot}")
                    for nh in range(2):
                        sl = slice(nh * 512, (nh + 1) * 512)
                        for g in range(2):
                            nc.tensor.matmul(
                                ps[:, sl],
                                lhsT=w8[:, 2 * g:2 * g + 2, ot * P:(ot + 1) * P],
                                rhs=x8[:, 2 * g:2 * g + 2, sl],
                                start=(g == 0),
                                stop=(g == 1 and not with_bias),
                                perf_mode=DRM)
                        if with_bias:
                            nc.tensor.matmul(
                                ps[:, sl],
                                lhsT=bq8[:, :, ot * P:(ot + 1) * P],
                                rhs=on8[:, :, sl],
                                start=False, stop=True, perf_mode=DRM,
                                skip_group_check=True)
                    dst = qt[ot] if name == "q" else kt[ot]
                    copy_psum(eng, dst[:, :], ps[:, :])

                def proj_v(a):
                    ps = pp.tile([P, N], F32, tag="pp", name=f"pp_v{a}")
                    for jloc in range(2):
                        jt = 2 * a + jloc
                        sl = slice(jloc * 512, (jloc + 1) * 512)
                        for g in range(2):
                            nc.tensor.matmul(
                                ps[:, sl],
                                lhsT=x8[:, 2 * g:2 * g + 2, jt * P:(jt + 1) * P],
                                rhs=wv8[:, 2 * g:2 * g + 2, :],
                                start=(g == 0), stop=(g == 1), perf_mode=DRM)
                    psv = ps.rearrange("p (s h d) -> p s h d", s=2, h=NH)
                    nc.gpsimd.tensor_tensor(
                        vt[a][:, :, :, 0:HD], psv, psv, AOP.max)
                    nc.vector.memset(vt[a][:, :, :, HD:HD + 1], 1.0)

                proj_qk(wq8, 0, True, QK_COPY_ENG[("q", 0)], "q")
                proj_qk(wk8, 0, False, QK_COPY_ENG[("k", 0)], "k")
                for jt in range(2):
                    s_tile(0, 0, jt)
                    s_tile(0, 1, jt)
                proj_qk(wq8, 1, True, QK_COPY_ENG[("q", 1)], "q")
                proj_qk(wk8, 1, False, QK_COPY_ENG[("k", 1)], "k")
                for jt in range(2, JT):
                    s_tile(0, 0, jt)
                    s_tile(0, 1, jt)
                proj_qk(wq8, 2, True, QK_COPY_ENG[("q", 2)], "q")
                proj_qk(wk8, 2, False, QK_COPY_ENG[("k", 2)], "k")
                for a in range(CT):
                    proj_v(a)
                proj_qk(wq8, 3, True, QK_COPY_ENG[("q", 3)], "q")
                proj_qk(wk8, 3, False, QK_COPY_ENG[("k", 3)], "k")

                # late loads for the output stage
                nc.gpsimd.dma_start(out=wo8, in_=wo8d[:, :, :])
                nc.sync.dma_start(out=xr32, in_=xrd[:, :, :])

            def av_head(h, psO):
                po = psO.tile([HD + 1, N], F32, tag="psO", name=f"po_{h}")
                for a in range(CT):
                    for ih in range(2):
                        nc.tensor.matmul(
                            po[:, ih * 512:(ih + 1) * 512],
                            lhsT=vt[a][:, :, h, :],
                            rhs=p8[(a, h)][:, :, ih * 512:(ih + 1) * 512],
                            start=(a == 0), stop=(a == CT - 1),
                            perf_mode=DRM, skip_group_check=True)
                # denominator row -> broadcast across 64 partitions (gpsimd)
                dben = small.tile([HD, N], F32, tag="dben", name=f"db_{h}")
                nc.gpsimd.partition_broadcast(dben[:, :], po[HD:HD + 1, :])
                g, s = h // 4, (h // 2) % 2
                odd = h % 2 == 1
                if odd:
                    dst = small.tile([HD, N], F8, tag="osc", name=f"osc_{h}")
                    dst_ap = dst[:, :]
                else:
                    dst_ap = o8[g][0:HD, s, :]
                eng = nc.vector if NORM_ENG[h] == "V" else nc.gpsimd
                eng.tensor_tensor(dst_ap, po[0:HD, :], dben[:, :], AOP.divide)
                if odd:
                    nc.sync.dma_start(out=o8[g][HD:P, s, :], in_=dst_ap)

            def out_proj(ot, pool, tag):
                ps = pool.tile([P, N], F32, tag=tag, name=f"op_{ot}")
                for nh in range(2):
                    sl = slice(nh * 512, (nh + 1) * 512)
                    for g in range(2):
                        nc.tensor.matmul(
                            ps[:, sl],
                            lhsT=wo8[:, 2 * g:2 * g + 2, ot * P:(ot + 1) * P],
                            rhs=o8[g][:, :, sl],
                            start=(g == 0), stop=(g == 1),
                            perf_mode=DRM, skip_group_check=True)
                ob = obf.tile([P, N], F32, tag="ob", name=f"ob_{ot}")
                eng = nc.vector if RESID_ENG[ot] == "V" else nc.gpsimd
                eng.tensor_tensor(ob[:, :], ps[:, :], xr32[:, ot, :], AOP.add)
                nc.sync.dma_start(out=outr[ot], in_=ob)

            with tc.tile_pool(name="psO", bufs=2, space="PSUM") as psO:
                for jt in range(JT):
                    s_tile(1, 0, jt)
                    s_tile(1, 1, jt)
                av_head(0, psO)
                av_head(1, psO)
                for jt in range(JT):
                    s_tile(2, 0, jt)
                    s_tile(2, 1, jt)
                av_head(2, psO)
                av_head(3, psO)
                for jt in range(JT):
                    s_tile(3, 0, jt)
                    s_tile(3, 1, jt)
                av_head(4, psO)
                av_head(5, psO)
                av_head(7, psO)
                av_head(6, psO)
                out_proj(0, psS, "psS")
                out_proj(1, psS, "psS")
            with tc.tile_pool(name="po3", bufs=2, space="PSUM") as po3:
                out_proj(2, po3, "po3")
                out_proj(3, po3, "po3")
    return nc


_BF = ml_dtypes.bfloat16
_E4 = ml_dtypes.float8_e4m3


def _prep_maps(x, Wq, bq, Wk, bk, Wv, bv, Wo, bo):
    # plain numpy up front: inputs may arrive as jax device arrays and
    # transforming those would trigger on-device jax execution
    x, Wq, bq, Wk, bk, Wv, bv, Wo, bo = (
        np.asarray(a, dtype=np.float32)
        for a in (x, Wq, bq, Wk, bk, Wv, bv, Wo, bo)
    )
    B, C, H, W = x.shape
    xf = np.ascontiguousarray(x.reshape(B, C, H * W))

    def wtile(Wm):
        # [128, CT, DIM] with [p, t, o] = W[o, 128t + p], fp8
        return np.ascontiguousarray(
            Wm.T.reshape(CT, P, DIM).transpose(1, 0, 2)).astype(_E4)

    bo_p = bo + Wo @ bv  # bv folded through the output projection (exact)
    bq8 = np.zeros((1, 2, DIM), np.float32)
    bq8[0, 0, :] = 64.0 * bq
    on8 = np.zeros((1, 2, N), np.float32)
    on8[0, 0, :] = 1.0 / 64.0
    shared = {
        "wq8": wtile(Wq), "wk8": wtile(Wk), "wv8": wtile(Wv), "wo8": wtile(Wo),
        "bq8": bq8.astype(_E4), "on8": on8.astype(_E4),
    }
    in_maps = []
    for b in range(B):
        m = dict(shared)
        xb = xf[b]                                  # [C, N]
        m["x8"] = np.ascontiguousarray(
            xb.reshape(CT, P, N).transpose(1, 0, 2)).astype(_E4)
        m["xr32"] = np.ascontiguousarray(
            (xb + bo_p[:, None]).reshape(CT, P, N).transpose(1, 0, 2))
        in_maps.append(m)
    return in_maps


def kernel(x, Wq, bq, Wk, bk, Wv, bv, Wo, bo, _trace=False):
    from concourse.bass_utils import run_bass_kernel_spmd

    x = np.asarray(x)
    B, C, H, W = x.shape
    in_maps = _prep_maps(x, Wq, bq, Wk, bk, Wv, bv, Wo, bo)
    nc = build_nc()
    res = run_bass_kernel_spmd(nc, in_maps, core_ids=list(range(B)), trace=_trace)
    out = np.stack([res.results[b]["out"] for b in range(B)])
    out = out.reshape(B, C, H, W).astype(np.float32)
    if _trace:
        kernel.last_results = res
    return out


# revision 14
# speedup vs baseline: 1.1428x; 1.1310x over previous
"""Multi-head attention (dense_transformer) Trainium2 Bass kernel.

Problem: x[8, 512, 32, 32]; per-batch 1x1-conv QKV projections, 8-head
attention over N=H*W=1024 positions (head_dim 64), output projection,
residual. Sharding: data-parallel over batch B=8 across the 8 cores --
one batch element per core, no collectives.

Key design points (v2, fp8-DoubleRow rewrite):
  - All projection / AV / output matmuls run in fp8e4m3 DoubleRow mode:
    one instruction contracts two 128-deep k-tiles at 0.5 cycles/row,
    4x the throughput of the bf16 accumulation chains it replaces.
  - Bias algebra: bk is dropped outright (softmax is shift-invariant
    along j, so the K bias cancels exactly); bv is folded into bo on the
    host (bo' = bo + Wo@bv, exact since attention rows sum to 1); bo'
    rides in the precomputed residual tensor xr = x + bo'; bq is added
    by a tiny [1-partition] DoubleRow ones-row matmul into the Q psum.
    No per-element bias vector ops remain on the device.
  - Q/K stay bf16 for the S = K^T Q matmuls (contraction is only 64
    deep -- fp8 DoubleRow does not apply -- and bf16 runs at the same
    1 cycle/row while keeping S accurate).
  - The softmax exp of the [1024, 1024] S matrix per head (8.4M
    elements -- the single largest engine workload) is split between
    the Activation engine (exact exp -> fp8 out) and the Vector engine
    (a one-instruction Schraudolph bit-trick: int8(S*A + B) aliased as
    fp8e4m3 approximates exp(S/8) to ~3% RMS, well inside the 2e-2
    tolerance).
  - Softmax denominators come free as a 65th ones-column of V^T in the
    AV matmul; gpsimd broadcasts the psum denominator row across 64
    partitions and the normalization is a single tensor_tensor divide
    (no reciprocal ops, no DRAM bounce).
  - GPSIMD (Pool) engine carries V-copies, denominator broadcasts and
    part of the normalize/residual work under the 'proxy' library.
"""

import sys

if "/opt/trn_rl_repo" not in sys.path:
    sys.path.insert(0, "/opt/trn_rl_repo")

import numpy as np
import ml_dtypes

import concourse.bass as bass
import concourse.mybir as mybir
from concourse.tile import TileContext
from concourse import library_config

DIM = 512
NH = 8
HD = 64
N = 1024
P = 128
CT = DIM // P   # 4 c-tiles of 128 channels
JT = N // P     # 8 j-tiles of 128 positions
F32 = mybir.dt.float32
BF16 = mybir.dt.bfloat16
F8 = mybir.dt.float8e4
I8 = mybir.dt.int8
AOP = mybir.AluOpType
EXP = mybir.ActivationFunctionType.Exp
DRM = mybir.MatmulPerfMode.DoubleRow

# Schraudolph exp: fp8e4m3 byte ~= int8(S * SCH_A + SCH_B)  approximates
# exp(S * 0.125). B tuned numerically for truncating float->int casts.
SCH_A = 8.0 / np.log(2.0) * 0.125
SCH_B = 56.08

# Exp engine split: within each jt the two head-tiles go to DIFFERENT
# engines (h2=0 -> ACT, h2=1 -> DVE) so both engines chew in parallel on
# the two psS bufs. A few h2=1 tiles go to ACT to balance total load;
# pair 3 (the tail) splits cleanly 8/8 so both engines finish together.
EXP_DVE = ({(p, 1, jt) for p in range(3) for jt in range(8) if jt not in (3, 6)}
           | {(3, 1, jt) for jt in range(8)})

# engine for each Q/K psum->bf16 copy, by (tensor, ot)
QK_COPY_ENG = {("q", 0): "A", ("k", 0): "V", ("q", 1): "A", ("k", 1): "V",
               ("q", 2): "P", ("k", 2): "P", ("q", 3): "A", ("k", 3): "V"}
# engine for the normalize divide, per head (ACT cannot run tensor_tensor)
NORM_ENG = {0: "P", 1: "V", 2: "P", 3: "V", 4: "P", 5: "V", 6: "V", 7: "V"}
# engine for the residual add, per ot
RESID_ENG = {0: "P", 1: "V", 2: "P", 3: "V"}

# head -> (g, s, half) slot in the output-projection rhs. The host
# permutes Wo's input-channel order to match (see _prep_maps). Chosen so
# the tail heads h6/h7 land in lower halves (direct engine write, no
# partition-remap DMA on the critical tail); the remapped (half=1) heads
# h1/h3/h4/h5 all complete mid-kernel.
HEAD_SLOT = {0: (0, 0, 0), 1: (0, 0, 1), 2: (0, 1, 0), 3: (0, 1, 1),
             4: (1, 0, 1), 5: (1, 1, 1), 6: (1, 1, 0), 7: (1, 0, 0)}


class FixedTileContext(TileContext):
    """Works around a walrus/bass snapshot mismatch: this walrus build
    accepts only one sync-wait command per instruction, but Tile's wait
    assigner happily attaches several. After scheduling, excess waits on
    any instruction are peeled off onto same-engine NOPs inserted right
    before it (same blocking semantics: the engine executes in order)."""

    MAX_WAITS = 1
    MAX_WAITS_DATA = 1
    _wsplit_ctr = 0

    def _split_sync_waits(self):
        seq_only = mybir.SEQUENCER_ONLY_OPCODES
        for fn in self.nc.m.functions:
            for blk in fn.blocks:
                insts = list(blk.instructions)
                out = []
                for inst in insts:
                    si = inst.sync_info
                    limit = (
                        self.MAX_WAITS
                        if inst.opcode in seq_only
                        else self.MAX_WAITS_DATA
                    )
                    if si is not None and len(si.on_wait) > limit:
                        waits = list(si.on_wait)
                        movers = waits[:-limit]
                        keep = waits[-limit:]
                        del si.on_wait[:]
                        for w in keep:
                            si.on_wait.append(w)
                        for w in movers:
                            FixedTileContext._wsplit_ctr += 1
                            nop = mybir.InstNoOp(
                                name=f"wsplit-{FixedTileContext._wsplit_ctr}",
                                ins=[],
                                outs=[],
                            )
                            nop.engine = inst.engine
                            nop.sync_info = mybir.SyncInfo(on_wait=[w], on_update=[])
                            out.append(nop)
                    out.append(inst)
                if len(out) != len(insts):
                    del blk.instructions[:]
                    for i in out:
                        blk.add_instruction(i)

    split_on_exit = True

    def __exit__(self, *exc):
        ret = super().__exit__(*exc)
        if exc[0] is None and self.split_on_exit:
            self._split_sync_waits()
        return ret


def build_nc(split_waits=True):
    nc = bass.Bass()

    x8d = nc.dram_tensor("x8", [P, CT, N], F8, kind="ExternalInput")
    xrd = nc.dram_tensor("xr32", [P, CT, N], F32, kind="ExternalInput")
    wq8d = nc.dram_tensor("wq8", [P, CT, DIM], F8, kind="ExternalInput")
    wk8d = nc.dram_tensor("wk8", [P, CT, DIM], F8, kind="ExternalInput")
    wv8d = nc.dram_tensor("wv8", [P, CT, DIM], F8, kind="ExternalInput")
    wo8d = nc.dram_tensor("wo8", [P, CT, DIM], F8, kind="ExternalInput")
    bq8d = nc.dram_tensor("bq8", [1, 2, DIM], F8, kind="ExternalInput")
    on8d = nc.dram_tensor("on8", [1, 2, N], F8, kind="ExternalInput")
    outd = nc.dram_tensor("out", [DIM, N], F32, kind="ExternalOutput")
    outr = outd.rearrange("(t p) n -> t p n", p=P)

    FixedTileContext.split_on_exit = split_waits
    with FixedTileContext(nc) as tc:
        with (
            tc.tile_pool(name="persist", bufs=1) as persist,
            tc.tile_pool(name="small", bufs=2) as small,
            tc.tile_pool(name="obf", bufs=4) as obf,
            tc.tile_pool(name="psS", bufs=2, space="PSUM") as psS,
        ):
            nc.gpsimd.load_library(library_config.proxy)

            # persistent SBUF tensors
            x8 = persist.tile([P, CT, N], F8, tag="x8", name="x8")
            xr32 = persist.tile([P, CT, N], F32, tag="xr32", name="xr32")
            wq8 = persist.tile([P, CT, DIM], F8, tag="wq8", name="wq8")
            wk8 = persist.tile([P, CT, DIM], F8, tag="wk8", name="wk8")
            wv8 = persist.tile([P, CT, DIM], F8, tag="wv8", name="wv8")
            wo8 = persist.tile([P, CT, DIM], F8, tag="wo8", name="wo8")
            bq8 = persist.tile([1, 2, DIM], F8, tag="bq8", name="bq8")
            on8 = persist.tile([1, 2, N], F8, tag="on8", name="on8")
            qt = [persist.tile([P, N], BF16, tag=f"q_{t}", name=f"q_{t}")
                  for t in range(CT)]
            kt = [persist.tile([P, N], BF16, tag=f"k_{t}", name=f"k_{t}")
                  for t in range(CT)]
            vt = [persist.tile([P, 2, NH, HD + 1], F8, tag=f"v_{a}",
                               name=f"v_{a}") for a in range(CT)]
            p8 = {}
            for a in range(CT):
                for h in range(NH):
                    p8[(a, h)] = persist.tile(
                        [P, 2, N], F8, tag=f"p8_{a}_{h}", name=f"p8_{a}_{h}")
            o8 = [persist.tile([P, 2, N], F8, tag=f"o8_{g}", name=f"o8_{g}")
                  for g in range(2)]

            def copy_psum(eng, out_ap, in_ap):
                if eng == "A":
                    nc.scalar.copy(out_ap, in_ap)
                elif eng == "V":
                    nc.vector.tensor_copy(out_ap, in_ap)
                else:
                    nc.gpsimd.tensor_tensor(out_ap, in_ap, in_ap, AOP.max)

            def exp_op(key, dst_ap, src_ap):
                if key in EXP_DVE:
                    nc.vector.tensor_scalar(dst_ap.bitcast(I8), src_ap,
                                            SCH_A, SCH_B, AOP.mult, AOP.add)
                else:
                    nc.scalar.activation(dst_ap, src_ap, EXP, scale=0.125)

            def s_tile(p, h2, jt):
                h = 2 * p + h2
                t = psS.tile([P, N], F32, tag="psS", name=f"s_{p}_{h2}_{jt}")
                for ih in range(2):
                    nc.tensor.matmul(
                        t[:, ih * 512:(ih + 1) * 512],
                        lhsT=kt[p][64 * h2:64 * h2 + 64, jt * P:(jt + 1) * P],
                        rhs=qt[p][64 * h2:64 * h2 + 64, ih * 512:(ih + 1) * 512],
                        start=True, stop=True)
                exp_op((p, h2, jt), p8[(jt // 2, h)][:, jt % 2, :], t)

            with tc.tile_pool(name="pp", bufs=2, space="PSUM") as pp:
                # ---- input DMAs ---- (ACT queue kept clear for copies/exps)
                nc.sync.dma_start(out=wq8, in_=wq8d[:, :, :])
                nc.sync.dma_start(out=x8[:, 0:2, :], in_=x8d[:, 0:2, :])
                nc.gpsimd.dma_start(out=x8[:, 2:4, :], in_=x8d[:, 2:4, :])
                nc.sync.dma_start(out=wk8, in_=wk8d[:, :, :])
                nc.scalar.dma_start(out=bq8, in_=bq8d[:, :, :])
                nc.scalar.dma_start(out=on8, in_=on8d[:, :, :])
                nc.sync.dma_start(out=wv8, in_=wv8d[:, :, :])

                # warm the exp table on ACT before the first real exp
                warm = small.tile([1, 8], F32, tag="warm", name="warm")
                nc.vector.memset(warm, 0.0)
                nc.scalar.activation(warm, warm, EXP)

                def proj_qk(w8, ot, with_bias, eng, name):
                    ps = pp.tile([P, N], F32, tag="pp", name=f"pp_{name}{ot}")
                    for nh in range(2):
                        sl = slice(nh * 512, (nh + 1) * 512)
                        for g in range(2):
                            nc.tensor.matmul(
                                ps[:, sl],
                                lhsT=w8[:, 2 * g:2 * g + 2, ot * P:(ot + 1) * P],
                                rhs=x8[:, 2 * g:2 * g + 2, sl],
                                start=(g == 0),
                                stop=(g == 1 and not with_bias),
                                perf_mode=DRM)
                        if with_bias:
                            nc.tensor.matmul(
                                ps[:, sl],
                                lhsT=bq8[:, :, ot * P:(ot + 1) * P],
                                rhs=on8[:, :, sl],
                                start=False, stop=True, perf_mode=DRM,
                                skip_group_check=True)
                    dst = qt[ot] if name == "q" else kt[ot]
                    copy_psum(eng, dst[:, :], ps[:, :])

                def proj_v(a):
                    ps = pp.tile([P, N], F32, tag="pp", name=f"pp_v{a}")
                    for jloc in range(2):
                        jt = 2 * a + jloc
                        sl = slice(jloc * 512, (jloc + 1) * 512)
                        for g in range(2):
                            nc.tensor.matmul(
                                ps[:, sl],
                                lhsT=x8[:, 2 * g:2 * g + 2, jt * P:(jt + 1) * P],
                                rhs=wv8[:, 2 * g:2 * g + 2, :],
                                start=(g == 0), stop=(g == 1), perf_mode=DRM)
                    psv = ps.rearrange("p (s h d) -> p s h d", s=2, h=NH)
                    nc.gpsimd.tensor_tensor(
                        vt[a][:, :, :, 0:HD], psv, psv, AOP.max)
                    nc.vector.memset(vt[a][:, :, :, HD:HD + 1], 1.0)

                proj_qk(wq8, 0, True, QK_COPY_ENG[("q", 0)], "q")
                proj_qk(wk8, 0, False, QK_COPY_ENG[("k", 0)], "k")
                for jt in range(2):
                    s_tile(0, 0, jt)
                    s_tile(0, 1, jt)
                proj_qk(wq8, 1, True, QK_COPY_ENG[("q", 1)], "q")
                proj_qk(wk8, 1, False, QK_COPY_ENG[("k", 1)], "k")
                for jt in range(2, JT):
                    s_tile(0, 0, jt)
                    s_tile(0, 1, jt)
                proj_qk(wq8, 2, True, QK_COPY_ENG[("q", 2)], "q")
                proj_qk(wk8, 2, False, QK_COPY_ENG[("k", 2)], "k")
                for a in range(CT):
                    proj_v(a)
                proj_qk(wq8, 3, True, QK_COPY_ENG[("q", 3)], "q")
                proj_qk(wk8, 3, False, QK_COPY_ENG[("k", 3)], "k")

                # late loads for the output stage
                nc.gpsimd.dma_start(out=wo8, in_=wo8d[:, :, :])
                nc.sync.dma_start(out=xr32, in_=xrd[:, :, :])

            def av_head(h, psO):
                po = psO.tile([HD + 1, N], F32, tag="psO", name=f"po_{h}")
                for a in range(CT):
                    for ih in range(2):
                        nc.tensor.matmul(
                            po[:, ih * 512:(ih + 1) * 512],
                            lhsT=vt[a][:, :, h, :],
                            rhs=p8[(a, h)][:, :, ih * 512:(ih + 1) * 512],
                            start=(a == 0), stop=(a == CT - 1),
                            perf_mode=DRM, skip_group_check=True)
                # denominator row -> broadcast across 64 partitions (gpsimd)
                dben = small.tile([HD, N], F32, tag="dben", name=f"db_{h}")
                nc.gpsimd.partition_broadcast(dben[:, :], po[HD:HD + 1, :])
                g, s, half = HEAD_SLOT[h]
                odd = half == 1
                if odd:
                    dst = small.tile([HD, N], F8, tag="osc", name=f"osc_{h}")
                    dst_ap = dst[:, :]
                else:
                    dst_ap = o8[g][0:HD, s, :]
                eng = nc.vector if NORM_ENG[h] == "V" else nc.gpsimd
                eng.tensor_tensor(dst_ap, po[0:HD, :], dben[:, :], AOP.divide)
                if odd:
                    nc.sync.dma_start(out=o8[g][HD:P, s, :], in_=dst_ap)

            def out_proj(ot, pool, tag):
                ps = pool.tile([P, N], F32, tag=tag, name=f"op_{ot}")
                eng = nc.vector if RESID_ENG[ot] == "V" else nc.gpsimd
                for nh in range(2):
                    sl = slice(nh * 512, (nh + 1) * 512)
                    for g in range(2):
                        nc.tensor.matmul(
                            ps[:, sl],
                            lhsT=wo8[:, 2 * g:2 * g + 2, ot * P:(ot + 1) * P],
                            rhs=o8[g][:, :, sl],
                            start=(g == 0), stop=(g == 1),
                            perf_mode=DRM, skip_group_check=True)
                # residual + writeback per half: finer tail staggering
                for nh in range(2):
                    sl = slice(nh * 512, (nh + 1) * 512)
                    ob = obf.tile([P, 512], F32, tag="ob", name=f"ob_{ot}_{nh}")
                    eng.tensor_tensor(ob[:, :], ps[:, sl], xr32[:, ot, sl],
                                      AOP.add)
                    nc.sync.dma_start(out=outr[ot][:, sl], in_=ob)

            with tc.tile_pool(name="psO", bufs=2, space="PSUM") as psO:
                for jt in range(JT):
                    s_tile(1, 0, jt)
                    s_tile(1, 1, jt)
                av_head(0, psO)
                av_head(1, psO)
                for jt in range(JT):
                    s_tile(2, 0, jt)
                    s_tile(2, 1, jt)
                av_head(2, psO)
                av_head(3, psO)
                for jt in range(JT):
                    s_tile(3, 1, jt)
                    s_tile(3, 0, jt)
                av_head(4, psO)
                av_head(5, psO)
                av_head(7, psO)
                av_head(6, psO)
                out_proj(0, psS, "psS")
                out_proj(1, psS, "psS")
            with tc.tile_pool(name="po3", bufs=2, space="PSUM") as po3:
                out_proj(2, po3, "po3")
                out_proj(3, po3, "po3")
    return nc


_BF = ml_dtypes.bfloat16
_E4 = ml_dtypes.float8_e4m3


def _prep_maps(x, Wq, bq, Wk, bk, Wv, bv, Wo, bo):
    # plain numpy up front: inputs may arrive as jax device arrays and
    # transforming those would trigger on-device jax execution
    x, Wq, bq, Wk, bk, Wv, bv, Wo, bo = (
        np.asarray(a, dtype=np.float32)
        for a in (x, Wq, bq, Wk, bk, Wv, bv, Wo, bo)
    )
    B, C, H, W = x.shape
    xf = np.ascontiguousarray(x.reshape(B, C, H * W))

    def wtile(Wm):
        # [128, CT, DIM] with [p, t, o] = W[o, 128t + p], fp8
        return np.ascontiguousarray(
            Wm.T.reshape(CT, P, DIM).transpose(1, 0, 2)).astype(_E4)

    # Wo's input channels are permuted to match the device's o8 head-slot
    # layout (HEAD_SLOT): channel 128*(2g+s) + 64*half + d <- head H's d.
    cperm = np.zeros(DIM, np.int64)
    for h, (g, s, half) in HEAD_SLOT.items():
        base = P * (2 * g + s) + HD * half
        cperm[base:base + HD] = HD * h + np.arange(HD)
    Wo_p = Wo[:, cperm]

    bo_p = bo + Wo @ bv  # bv folded through the output projection (exact)
    bq8 = np.zeros((1, 2, DIM), np.float32)
    bq8[0, 0, :] = 64.0 * bq
    on8 = np.zeros((1, 2, N), np.float32)
    on8[0, 0, :] = 1.0 / 64.0
    shared = {
        "wq8": wtile(Wq), "wk8": wtile(Wk), "wv8": wtile(Wv),
        "wo8": wtile(Wo_p), "bq8": bq8.astype(_E4), "on8": on8.astype(_E4),
    }
    in_maps = []
    for b in range(B):
        m = dict(shared)
        xb = xf[b]                                  # [C, N]
        m["x8"] = np.ascontiguousarray(
            xb.reshape(CT, P, N).transpose(1, 0, 2)).astype(_E4)
        m["xr32"] = np.ascontiguousarray(
            (xb + bo_p[:, None]).reshape(CT, P, N).transpose(1, 0, 2))
        in_maps.append(m)
    return in_maps


def kernel(x, Wq, bq, Wk, bk, Wv, bv, Wo, bo, _trace=False):
    from concourse.bass_utils import run_bass_kernel_spmd

    x = np.asarray(x)
    B, C, H, W = x.shape
    in_maps = _prep_maps(x, Wq, bq, Wk, bk, Wv, bv, Wo, bo)
    nc = build_nc()
    res = run_bass_kernel_spmd(nc, in_maps, core_ids=list(range(B)), trace=_trace)
    out = np.stack([res.results[b]["out"] for b in range(B)])
    out = out.reshape(B, C, H, W).astype(np.float32)
    if _trace:
        kernel.last_results = res
    return out


# revision 20
# speedup vs baseline: 1.3105x; 1.1467x over previous
"""Multi-head attention (dense_transformer) Trainium2 Bass kernel.

Problem: x[8, 512, 32, 32]; per-batch 1x1-conv QKV projections, 8-head
attention over N=H*W=1024 positions (head_dim 64), output projection,
residual. Sharding: data-parallel over batch B=8 across the 8 cores --
one batch element per core, no collectives.

Key design points (v2, fp8-DoubleRow rewrite):
  - All projection / AV / output matmuls run in fp8e4m3 DoubleRow mode:
    one instruction contracts two 128-deep k-tiles at 0.5 cycles/row,
    4x the throughput of the bf16 accumulation chains it replaces.
  - Bias algebra: bk is dropped outright (softmax is shift-invariant
    along j, so the K bias cancels exactly); bv is folded into bo on the
    host (bo' = bo + Wo@bv, exact since attention rows sum to 1); bo'
    rides in the precomputed residual tensor xr = x + bo'; bq is added
    by a tiny [1-partition] DoubleRow ones-row matmul into the Q psum.
    No per-element bias vector ops remain on the device.
  - Q/K stay bf16 for the S = K^T Q matmuls (contraction is only 64
    deep -- fp8 DoubleRow does not apply -- and bf16 runs at the same
    1 cycle/row while keeping S accurate).
  - The softmax exp of the [1024, 1024] S matrix per head (8.4M
    elements -- the single largest engine workload) is split between
    the Activation engine (exact exp -> fp8 out) and the Vector engine
    (a one-instruction Schraudolph bit-trick: int8(S*A + B) aliased as
    fp8e4m3 approximates exp(S/8) to ~3% RMS, well inside the 2e-2
    tolerance).
  - Softmax denominators come free as a 65th ones-column of V^T in the
    AV matmul; gpsimd broadcasts the psum denominator row across 64
    partitions and the normalization is a single tensor_tensor divide
    (no reciprocal ops, no DRAM bounce).
  - GPSIMD (Pool) engine carries V-copies, denominator broadcasts and
    part of the normalize/residual work under the 'proxy' library.
"""

import sys

if "/opt/trn_rl_repo" not in sys.path:
    sys.path.insert(0, "/opt/trn_rl_repo")

import numpy as np
import ml_dtypes

import concourse.bass as bass
import concourse.mybir as mybir
from concourse.tile import TileContext
from concourse import library_config

DIM = 512
NH = 8
HD = 64
N = 1024
P = 128
CT = DIM // P   # 4 c-tiles of 128 channels
JT = N // P     # 8 j-tiles of 128 positions
F32 = mybir.dt.float32
BF16 = mybir.dt.bfloat16
F8 = mybir.dt.float8e4
I8 = mybir.dt.int8
AOP = mybir.AluOpType
EXP = mybir.ActivationFunctionType.Exp
DRM = mybir.MatmulPerfMode.DoubleRow

# Schraudolph exp: fp8e4m3 byte ~= int8(S * SCH_A + SCH_B)  approximates
# exp(S * 0.125). B tuned numerically for truncating float->int casts.
SCH_A = 8.0 / np.log(2.0) * 0.125
SCH_B = 56.08

# Exp engine split: within each jt the two head-tiles go to DIFFERENT
# engines (h2=0 -> ACT, h2=1 -> DVE) so both engines chew in parallel on
# the two psS bufs. A few h2=1 tiles go to ACT to balance total load;
# pair 3 (the tail) splits cleanly 8/8 so both engines finish together.
EXP_DVE = ({(p, 1, jt) for p in range(3) for jt in range(8) if jt not in (3, 6)}
           | {(3, 1, jt) for jt in range(8)})

# engine for each Q/K psum->bf16 copy, by (tensor, ot)
QK_COPY_ENG = {("q", 0): "A", ("k", 0): "V", ("q", 1): "A", ("k", 1): "V",
               ("q", 2): "P", ("k", 2): "P", ("q", 3): "A", ("k", 3): "V"}
# engine for the normalize divide, per head (ACT cannot run tensor_tensor)
NORM_ENG = {0: "P", 1: "V", 2: "P", 3: "V", 4: "P", 5: "V", 6: "V", 7: "V"}
# engine for the residual add, per ot
RESID_ENG = {0: "P", 1: "V", 2: "P", 3: "V"}

# head -> (g, s, half) slot in the output-projection rhs. The host
# permutes Wo's input-channel order to match (see _prep_maps). Chosen so
# the tail heads h6/h7 land in lower halves (direct engine write, no
# partition-remap DMA on the critical tail); the remapped (half=1) heads
# h1/h3/h4/h5 all complete mid-kernel.
HEAD_SLOT = {0: (0, 0, 0), 1: (0, 0, 1), 2: (0, 1, 0), 3: (0, 1, 1),
             4: (1, 0, 1), 5: (1, 1, 1), 6: (1, 1, 0), 7: (1, 0, 0)}


class FixedTileContext(TileContext):
    """Works around a walrus/bass snapshot mismatch: this walrus build
    accepts only one sync-wait command per instruction, but Tile's wait
    assigner happily attaches several. After scheduling, excess waits on
    any instruction are peeled off onto same-engine NOPs inserted right
    before it (same blocking semantics: the engine executes in order)."""

    MAX_WAITS = 1
    MAX_WAITS_DATA = 1
    _wsplit_ctr = 0

    def _split_sync_waits(self):
        seq_only = mybir.SEQUENCER_ONLY_OPCODES
        for fn in self.nc.m.functions:
            for blk in fn.blocks:
                insts = list(blk.instructions)
                out = []
                for inst in insts:
                    si = inst.sync_info
                    limit = (
                        self.MAX_WAITS
                        if inst.opcode in seq_only
                        else self.MAX_WAITS_DATA
                    )
                    if si is not None and len(si.on_wait) > limit:
                        waits = list(si.on_wait)
                        movers = waits[:-limit]
                        keep = waits[-limit:]
                        del si.on_wait[:]
                        for w in keep:
                            si.on_wait.append(w)
                        for w in movers:
                            FixedTileContext._wsplit_ctr += 1
                            nop = mybir.InstNoOp(
                                name=f"wsplit-{FixedTileContext._wsplit_ctr}",
                                ins=[],
                                outs=[],
                            )
                            nop.engine = inst.engine
                            nop.sync_info = mybir.SyncInfo(on_wait=[w], on_update=[])
                            out.append(nop)
                    out.append(inst)
                if len(out) != len(insts):
                    del blk.instructions[:]
                    for i in out:
                        blk.add_instruction(i)

    split_on_exit = True

    def __exit__(self, *exc):
        ret = super().__exit__(*exc)
        if exc[0] is None and self.split_on_exit:
            self._split_sync_waits()
        return ret


def build_nc(split_waits=True):
    nc = bass.Bass()

    x8d = nc.dram_tensor("x8", [P, CT, N], F8, kind="ExternalInput")
    xrd = nc.dram_tensor("xr32", [P, CT, N], F32, kind="ExternalInput")
    wq8d = nc.dram_tensor("wq8", [P, CT, DIM], F8, kind="ExternalInput")
    wk8d = nc.dram_tensor("wk8", [P, CT, DIM], F8, kind="ExternalInput")
    wv8d = nc.dram_tensor("wv8", [P, CT, DIM], F8, kind="ExternalInput")
    wo8d = nc.dram_tensor("wo8", [P, CT, DIM], F8, kind="ExternalInput")
    bq8d = nc.dram_tensor("bq8", [1, 2, DIM], F8, kind="ExternalInput")
    on8d = nc.dram_tensor("on8", [1, 2, N], F8, kind="ExternalInput")
    outd = nc.dram_tensor("out", [DIM, N], F32, kind="ExternalOutput")
    outr = outd.rearrange("(t p) n -> t p n", p=P)

    FixedTileContext.split_on_exit = split_waits
    with FixedTileContext(nc) as tc:
        with (
            tc.tile_pool(name="persist", bufs=1) as persist,
            tc.tile_pool(name="small", bufs=3) as small,
            tc.tile_pool(name="obf", bufs=4) as obf,
            tc.tile_pool(name="psS", bufs=2, space="PSUM") as psS,
        ):
            nc.gpsimd.load_library(library_config.proxy)

            # persistent SBUF tensors
            x8 = persist.tile([P, CT, N], F8, tag="x8", name="x8")
            xr32 = persist.tile([P, CT, N], F32, tag="xr32", name="xr32")
            wq8 = persist.tile([P, CT, DIM], F8, tag="wq8", name="wq8")
            wk8 = persist.tile([P, CT, DIM], F8, tag="wk8", name="wk8")
            wv8 = persist.tile([P, CT, DIM], F8, tag="wv8", name="wv8")
            wo8 = persist.tile([P, CT, DIM], F8, tag="wo8", name="wo8")
            bq8 = persist.tile([1, 2, DIM], F8, tag="bq8", name="bq8")
            on8 = persist.tile([1, 2, N], F8, tag="on8", name="on8")
            qt = [persist.tile([P, N], BF16, tag=f"q_{t}", name=f"q_{t}")
                  for t in range(CT)]
            kt = [persist.tile([P, N], BF16, tag=f"k_{t}", name=f"k_{t}")
                  for t in range(CT)]
            vt = [persist.tile([P, 2, NH, HD + 1], F8, tag=f"v_{a}",
                               name=f"v_{a}") for a in range(CT)]
            p8 = {}
            for a in range(CT):
                for h in range(NH):
                    p8[(a, h)] = persist.tile(
                        [P, 2, N], F8, tag=f"p8_{a}_{h}", name=f"p8_{a}_{h}")
            o8 = [persist.tile([P, 2, N], F8, tag=f"o8_{g}", name=f"o8_{g}")
                  for g in range(2)]

            def copy_psum(eng, out_ap, in_ap):
                if eng == "A":
                    nc.scalar.copy(out_ap, in_ap)
                elif eng == "V":
                    nc.vector.tensor_copy(out_ap, in_ap)
                else:
                    nc.gpsimd.tensor_tensor(out_ap, in_ap, in_ap, AOP.max)

            def exp_op(key, dst_ap, src_ap):
                if key in EXP_DVE:
                    nc.vector.tensor_scalar(dst_ap.bitcast(I8), src_ap,
                                            SCH_A, SCH_B, AOP.mult, AOP.add)
                else:
                    nc.scalar.activation(dst_ap, src_ap, EXP, scale=0.125)

            # S tiles cycle through 3 slots (2 in psS + 1 in psS2 once the
            # projection pool has closed) so both exp engines always have a
            # tile in flight and a third is being refilled by the PE.
            s_slot = [0]

            def s_tile(p, h2, jt, pools):
                h = 2 * p + h2
                pool, ptag = pools[s_slot[0] % len(pools)]
                s_slot[0] += 1
                t = pool.tile([P, N], F32, tag=ptag, name=f"s_{p}_{h2}_{jt}")
                for ih in range(2):
                    nc.tensor.matmul(
                        t[:, ih * 512:(ih + 1) * 512],
                        lhsT=kt[p][64 * h2:64 * h2 + 64, jt * P:(jt + 1) * P],
                        rhs=qt[p][64 * h2:64 * h2 + 64, ih * 512:(ih + 1) * 512],
                        start=True, stop=True)
                exp_op((p, h2, jt), p8[(jt // 2, h)][:, jt % 2, :], t)

            with tc.tile_pool(name="pp", bufs=2, space="PSUM") as pp:
                # ---- input DMAs ---- (ACT queue kept clear for copies/exps)
                nc.sync.dma_start(out=wq8, in_=wq8d[:, :, :])
                nc.sync.dma_start(out=x8[:, 0:2, :], in_=x8d[:, 0:2, :])
                nc.gpsimd.dma_start(out=x8[:, 2:4, :], in_=x8d[:, 2:4, :])
                nc.sync.dma_start(out=wk8, in_=wk8d[:, :, :])
                nc.scalar.dma_start(out=bq8, in_=bq8d[:, :, :])
                nc.scalar.dma_start(out=on8, in_=on8d[:, :, :])
                nc.sync.dma_start(out=wv8, in_=wv8d[:, :, :])

                # warm the exp table on ACT before the first real exp
                warm = small.tile([1, 8], F32, tag="warm", name="warm")
                nc.vector.memset(warm, 0.0)
                nc.scalar.activation(warm, warm, EXP)

                def proj_qk(w8, ot, with_bias, eng, name):
                    ps = pp.tile([P, N], F32, tag="pp", name=f"pp_{name}{ot}")
                    for nh in range(2):
                        sl = slice(nh * 512, (nh + 1) * 512)
                        for g in range(2):
                            nc.tensor.matmul(
                                ps[:, sl],
                                lhsT=w8[:, 2 * g:2 * g + 2, ot * P:(ot + 1) * P],
                                rhs=x8[:, 2 * g:2 * g + 2, sl],
                                start=(g == 0),
                                stop=(g == 1 and not with_bias),
                                perf_mode=DRM)
                        if with_bias:
                            nc.tensor.matmul(
                                ps[:, sl],
                                lhsT=bq8[:, :, ot * P:(ot + 1) * P],
                                rhs=on8[:, :, sl],
                                start=False, stop=True, perf_mode=DRM,
                                skip_group_check=True)
                    dst = qt[ot] if name == "q" else kt[ot]
                    copy_psum(eng, dst[:, :], ps[:, :])

                def proj_v(a):
                    ps = pp.tile([P, N], F32, tag="pp", name=f"pp_v{a}")
                    for jloc in range(2):
                        jt = 2 * a + jloc
                        sl = slice(jloc * 512, (jloc + 1) * 512)
                        for g in range(2):
                            nc.tensor.matmul(
                                ps[:, sl],
                                lhsT=x8[:, 2 * g:2 * g + 2, jt * P:(jt + 1) * P],
                                rhs=wv8[:, 2 * g:2 * g + 2, :],
                                start=(g == 0), stop=(g == 1), perf_mode=DRM)
                    psv = ps.rearrange("p (s h d) -> p s h d", s=2, h=NH)
                    nc.gpsimd.tensor_tensor(
                        vt[a][:, :, :, 0:HD], psv, psv, AOP.max)
                    nc.vector.memset(vt[a][:, :, :, HD:HD + 1], 1.0)

                proj_qk(wq8, 0, True, QK_COPY_ENG[("q", 0)], "q")
                proj_qk(wk8, 0, False, QK_COPY_ENG[("k", 0)], "k")
                for jt in range(2):
                    s_tile(0, 0, jt, [(psS, "psS")])
                    s_tile(0, 1, jt, [(psS, "psS")])
                proj_qk(wq8, 1, True, QK_COPY_ENG[("q", 1)], "q")
                proj_qk(wk8, 1, False, QK_COPY_ENG[("k", 1)], "k")
                for jt in range(2, JT):
                    s_tile(0, 0, jt, [(psS, "psS")])
                    s_tile(0, 1, jt, [(psS, "psS")])
                proj_qk(wq8, 2, True, QK_COPY_ENG[("q", 2)], "q")
                proj_qk(wk8, 2, False, QK_COPY_ENG[("k", 2)], "k")
                for a in range(CT):
                    proj_v(a)
                proj_qk(wq8, 3, True, QK_COPY_ENG[("q", 3)], "q")
                proj_qk(wk8, 3, False, QK_COPY_ENG[("k", 3)], "k")

                # late loads for the output stage
                nc.gpsimd.dma_start(out=wo8, in_=wo8d[:, :, :])
                nc.sync.dma_start(out=xr32, in_=xrd[:, :, :])

            def av_head(h, psO):
                g, s, half = HEAD_SLOT[h]
                odd = half == 1
                osc = None
                if odd:
                    osc = small.tile([HD, N], F8, tag="osc", name=f"osc_{h}")
                eng = nc.vector if NORM_ENG[h] == "V" else nc.gpsimd
                for ih in range(2):
                    sl = slice(ih * 512, (ih + 1) * 512)
                    po = psO.tile([HD + 1, 512], F32, tag="psO",
                                  name=f"po_{h}_{ih}")
                    for a in range(CT):
                        nc.tensor.matmul(
                            po, lhsT=vt[a][:, :, h, :],
                            rhs=p8[(a, h)][:, :, sl],
                            start=(a == 0), stop=(a == CT - 1),
                            perf_mode=DRM, skip_group_check=True)
                    # denominator row -> 64 partitions (gpsimd broadcast),
                    # then a single divide normalizes and casts to fp8
                    dben = small.tile([HD, 512], F32, tag="dben",
                                      name=f"db_{h}_{ih}")
                    nc.gpsimd.partition_broadcast(dben[:, :], po[HD:HD + 1, :])
                    dst_ap = osc[:, sl] if odd else o8[g][0:HD, s, sl]
                    eng.tensor_tensor(dst_ap, po[0:HD, :], dben[:, :],
                                      AOP.divide)
                if odd:
                    nc.sync.dma_start(out=o8[g][HD:P, s, :], in_=osc[:, :])

            def op_mm(ot, g, ps):
                # one g-layer of the output projection for both n-halves;
                # g-major emission lets the g0 layers run (and warm the PE
                # p-state) while the last heads' normalize is still going.
                for nh in range(2):
                    sl = slice(nh * 512, (nh + 1) * 512)
                    nc.tensor.matmul(
                        ps[:, sl],
                        lhsT=wo8[:, 2 * g:2 * g + 2, ot * P:(ot + 1) * P],
                        rhs=o8[g][:, :, sl],
                        start=(g == 0), stop=(g == 1),
                        perf_mode=DRM, skip_group_check=True)

            def op_tail(ot, ps):
                # residual + writeback per half: finer tail staggering
                eng = nc.vector if RESID_ENG[ot] == "V" else nc.gpsimd
                for nh in range(2):
                    sl = slice(nh * 512, (nh + 1) * 512)
                    ob = obf.tile([P, 512], F32, tag="ob", name=f"ob_{ot}_{nh}")
                    eng.tensor_tensor(ob[:, :], ps[:, sl], xr32[:, ot, sl],
                                      AOP.add)
                    nc.sync.dma_start(out=outr[ot][:, sl], in_=ob)

            with (
                tc.tile_pool(name="psS2", bufs=1, space="PSUM") as psS2,
                tc.tile_pool(name="psO", bufs=2, space="PSUM") as psO,
            ):
                pools3 = [(psS, "psS"), (psS, "psS"), (psS2, "psS2")]
                for jt in range(JT):
                    s_tile(1, 0, jt, pools3)
                    s_tile(1, 1, jt, pools3)
                av_head(0, psO)
                av_head(1, psO)
                for jt in range(JT):
                    s_tile(2, 0, jt, pools3)
                    s_tile(2, 1, jt, pools3)
                av_head(2, psO)
                av_head(3, psO)
                for jt in range(JT):
                    s_tile(3, 1, jt, pools3)
                    s_tile(3, 0, jt, pools3)
                av_head(4, psO)
                av_head(5, psO)
                av_head(7, psO)
                av_head(6, psO)
                ps0 = psS.tile([P, N], F32, tag="psS", name="op_0")
                ps1 = psS.tile([P, N], F32, tag="psS", name="op_1")
                op_mm(0, 0, ps0)
                op_mm(1, 0, ps1)
                op_mm(0, 1, ps0)
                op_mm(1, 1, ps1)
                op_tail(0, ps0)
                op_tail(1, ps1)
            with tc.tile_pool(name="po3", bufs=2, space="PSUM") as po3:
                ps2 = po3.tile([P, N], F32, tag="po3", name="op_2")
                ps3 = po3.tile([P, N], F32, tag="po3", name="op_3")
                op_mm(2, 0, ps2)
                op_mm(3, 0, ps3)
                op_mm(2, 1, ps2)
                op_mm(3, 1, ps3)
                op_tail(2, ps2)
                op_tail(3, ps3)
    return nc


_BF = ml_dtypes.bfloat16
_E4 = ml_dtypes.float8_e4m3


def _prep_maps(x, Wq, bq, Wk, bk, Wv, bv, Wo, bo):
    # plain numpy up front: inputs may arrive as jax device arrays and
    # transforming those would trigger on-device jax execution
    x, Wq, bq, Wk, bk, Wv, bv, Wo, bo = (
        np.asarray(a, dtype=np.float32)
        for a in (x, Wq, bq, Wk, bk, Wv, bv, Wo, bo)
    )
    B, C, H, W = x.shape
    xf = np.ascontiguousarray(x.reshape(B, C, H * W))

    def wtile(Wm):
        # [128, CT, DIM] with [p, t, o] = W[o, 128t + p], fp8
        return np.ascontiguousarray(
            Wm.T.reshape(CT, P, DIM).transpose(1, 0, 2)).astype(_E4)

    # Wo's input channels are permuted to match the device's o8 head-slot
    # layout (HEAD_SLOT): channel 128*(2g+s) + 64*half + d <- head H's d.
    cperm = np.zeros(DIM, np.int64)
    for h, (g, s, half) in HEAD_SLOT.items():
        base = P * (2 * g + s) + HD * half
        cperm[base:base + HD] = HD * h + np.arange(HD)
    Wo_p = Wo[:, cperm]

    bo_p = bo + Wo @ bv  # bv folded through the output projection (exact)
    bq8 = np.zeros((1, 2, DIM), np.float32)
    bq8[0, 0, :] = 64.0 * bq
    on8 = np.zeros((1, 2, N), np.float32)
    on8[0, 0, :] = 1.0 / 64.0
    shared = {
        "wq8": wtile(Wq), "wk8": wtile(Wk), "wv8": wtile(Wv),
        "wo8": wtile(Wo_p), "bq8": bq8.astype(_E4), "on8": on8.astype(_E4),
    }
    in_maps = []
    for b in range(B):
        m = dict(shared)
        xb = xf[b]                                  # [C, N]
        m["x8"] = np.ascontiguousarray(
            xb.reshape(CT, P, N).transpose(1, 0, 2)).astype(_E4)
        m["xr32"] = np.ascontiguousarray(
            (xb + bo_p[:, None]).reshape(CT, P, N).transpose(1, 0, 2))
        in_maps.append(m)
    return in_maps


def kernel(x, Wq, bq, Wk, bk, Wv, bv, Wo, bo, _trace=False):
    from concourse.bass_utils import run_bass_kernel_spmd

    x = np.asarray(x)
    B, C, H, W = x.shape
    in_maps = _prep_maps(x, Wq, bq, Wk, bk, Wv, bv, Wo, bo)
    nc = build_nc()
    res = run_bass_kernel_spmd(nc, in_maps, core_ids=list(range(B)), trace=_trace)
    out = np.stack([res.results[b]["out"] for b in range(B)])
    out = out.reshape(B, C, H, W).astype(np.float32)
    if _trace:
        kernel.last_results = res
    return out
